# revision 5
# baseline (speedup 1.0000x reference)
"""Self-contained Trainium2 kernel for nn_BaselineDynamics (LSTM + linear head).

kernel(**inputs) takes the FULL inputs (z_seq [2048,512,18], LSTM/linear
weights), shards the batch across 8 NeuronCores (pure data parallelism,
replicated weights), runs a Bass/Tile kernel per core, and returns the full
(z_pred, (h_n, c_n)) matching reference().

See lstm_tile_kernel docstring below for the device-side algorithm.
"""

import os
from contextlib import ExitStack

import numpy as np

import concourse.bass as bass
import concourse.bacc as bacc
import concourse.tile as tile
from concourse import mybir
from concourse._compat import with_exitstack
from concourse.bass_utils import run_bass_kernel_spmd

F32 = mybir.dt.float32

LATENT = 18
HID = 32
NB = 2048
T_FULL = 512
NCORES = 8
B_CORE = NB // NCORES  # 256

# torch gate order [i, f, g, o] -> device order [f, i, o, g]
# (f/o at slot base 0, i/g at slot base 32 so every elementwise op's two
# inputs share a partition base -- walrus requires equal bases for SB+SB ops)
PERM = np.concatenate(
    [np.arange(32, 64), np.arange(0, 32), np.arange(96, 128), np.arange(64, 96)]
)


def prep_weights(W_ih, W_hh, b_ih, b_hh, W_out, b_out):
    """Host-side weight folding.

    Returns lhsT matrices:
      w_z  [18, 128] : W_ih (permuted, g-rows doubled)
      w_hz [33, 128] : [2*W_hh (permuted, g-rows doubled); b_ih+b_hh (permuted, g-rows doubled)]
      w_zp [33, 32]  : [2*W_out; b_out] zero-padded to 32 columns
    """
    Wih_p = np.array(W_ih, dtype=np.float32)[PERM].copy()
    Whh_p = np.array(W_hh, dtype=np.float32)[PERM].copy()
    b_pv = (np.array(b_ih, dtype=np.float32) + np.array(b_hh, dtype=np.float32))[
        PERM
    ].copy()
    gsl = slice(96, 128)
    Wih_p[gsl] *= 2.0
    Whh_p[gsl] *= 2.0
    b_pv[gsl] *= 2.0
    Whh2 = 2.0 * Whh_p  # h = 2*h'
    w_z = np.ascontiguousarray(Wih_p.T)  # [18, 128]
    w_hz = np.ascontiguousarray(
        np.concatenate([Whh2.T, b_pv[None, :]], axis=0)
    )  # [33, 128]
    w_zp = np.concatenate(
        [
            (2.0 * np.array(W_out, dtype=np.float32)).T,
            np.array(b_out, dtype=np.float32)[None, :],
        ],
        axis=0,
    )  # [33, 18]
    w_zp = np.ascontiguousarray(
        np.concatenate([w_zp, np.zeros((HID + 1, 32 - LATENT), np.float32)], axis=1)
    )  # [33, 32] zero-padded so each PSUM slot is fully written
    return w_z, w_hz, w_zp


@with_exitstack
def lstm_tile_kernel(
    ctx: ExitStack, tc: tile.TileContext, outs, ins, T=T_FULL, B=B_CORE, M=2
):
    """Gate-major LSTM.  Per step t:
      psum_g = W_z.T @ z_t + W_hz.T @ [h'; 1]      (PE)
      S      = sigmoid(psum_g)                      (ACT; tanh folded via 2sig(2x)-1)
      u      = (S_g - 0.5) * S_i                    (GPSIMD)  = i*g/2
      tmp    = S_f * c'; c' = tmp + u               (DVE)
      Tc     = sigmoid(4*c')                        (ACT)     = (tanh(c)+1)/2
      h'     = (Tc - 0.5) * S_o  -> stack[0:32]     (DVE)     = h/2
      zp     = W_zp.T @ [h'; 1]                     (PE, 4 steps per PSUM tile)
    """
    nc = tc.nc
    zp_out, hc_out = outs
    zt_in, w_z_in, w_hz_in, w_zp_in = ins
    BC = B // M

    const_pool = ctx.enter_context(tc.tile_pool(name="const", bufs=1))
    state_pool = ctx.enter_context(tc.tile_pool(name="state", bufs=1))
    zpool = ctx.enter_context(tc.tile_pool(name="zin", bufs=6))
    spool = ctx.enter_context(tc.tile_pool(name="sig", bufs=3))
    vpool = ctx.enter_context(tc.tile_pool(name="vec", bufs=3))
    gpsum = ctx.enter_context(tc.tile_pool(name="gps", bufs=3, space="PSUM"))
    zpsum = ctx.enter_context(tc.tile_pool(name="zps", bufs=3, space="PSUM"))

    wz_sb = const_pool.tile([LATENT, 128], F32, tag="wz")
    nc.sync.dma_start(wz_sb[:], w_z_in[:])
    whz_sb = const_pool.tile([HID + 1, 128], F32, tag="whz")
    nc.sync.dma_start(whz_sb[:], w_hz_in[:])
    wzp_sb = const_pool.tile([HID + 1, 32], F32, tag="wzp")
    nc.sync.dma_start(wzp_sb[:], w_zp_in[:])

    stack = state_pool.tile([HID + 1, B], F32, tag="stack")  # 0:32 h', row 32 ones
    cp = state_pool.tile([HID, B], F32, tag="cp")
    nc.vector.memset(stack[0:HID, :], 0.0)
    nc.vector.memset(stack[HID : HID + 1, :], 1.0)
    nc.vector.memset(cp[:], 0.0)

    SIG = mybir.ActivationFunctionType.Sigmoid

    for t in range(T):
        z_t = zpool.tile([LATENT, B], F32, tag="z")
        nc.sync.dma_start(z_t[:], zt_in[t])

        ps = gpsum.tile([128, B], F32, tag="g")
        for m in range(M):
            cs = bass.ts(m, BC)
            nc.tensor.matmul(ps[:, cs], wz_sb[:], z_t[:, cs], start=True, stop=False)
            nc.tensor.matmul(ps[:, cs], whz_sb[:], stack[:, cs], start=False, stop=True)

        # S split into two base-0 tiles: s_lo rows {f, i}, s_hi rows {o, g}
        s_lo = spool.tile([64, B], F32, tag="Slo")
        s_hi = spool.tile([64, B], F32, tag="Shi")
        nc.scalar.activation(s_lo[:], ps[0:64, :], SIG)
        nc.scalar.activation(s_hi[:], ps[64:128, :], SIG)

        u = vpool.tile([HID, B], F32, tag="u")
        nc.vector.scalar_tensor_tensor(
            u[:], s_hi[32:64, :], 0.5, s_lo[32:64, :],
            op0=mybir.AluOpType.subtract, op1=mybir.AluOpType.mult,
        )
        tmp = vpool.tile([HID, B], F32, tag="tmp")
        nc.gpsimd.tensor_mul(tmp[:], s_lo[0:32, :], cp[:])
        nc.vector.tensor_add(cp[:], tmp[:], u[:])

        Tc = vpool.tile([HID, B], F32, tag="Tc")
        nc.scalar.activation(Tc[:], cp[:], SIG, scale=4.0)
        nc.vector.scalar_tensor_tensor(
            stack[0:HID, :], Tc[:], 0.5, s_hi[0:32, :],
            op0=mybir.AluOpType.subtract, op1=mybir.AluOpType.mult,
        )

        # zp for 4 consecutive steps packed into one PSUM tile at partition
        # slots 32*j, copied and DMA'd out once per 4 steps.
        j = t % 4
        if j == 0:
            zps = zpsum.tile([128, B], F32, tag="zp")
        for m in range(M):
            cs = bass.ts(m, BC)
            nc.tensor.matmul(
                zps[32 * j : 32 * j + 32, cs], wzp_sb[:], stack[:, cs],
                start=True, stop=True, tile_position=(0, 32 * j),
            )
        if j == 3 or t == T - 1:
            zsb = spool.tile([128, B], F32, tag="zsb")
            nc.vector.tensor_copy(zsb[: 32 * (j + 1), :], zps[: 32 * (j + 1), :])
            for jj in range(j + 1):
                nc.sync.dma_start(
                    zp_out[t - j + jj], zsb[32 * jj : 32 * jj + LATENT, :]
                )

    fin_h = vpool.tile([HID, B], F32, tag="fin")
    nc.scalar.mul(fin_h[:], stack[0:HID, :], 2.0)
    nc.sync.dma_start(hc_out[0], fin_h[:])
    fin_c = vpool.tile([HID, B], F32, tag="fin")
    nc.scalar.mul(fin_c[:], cp[:], 2.0)
    nc.sync.dma_start(hc_out[1], fin_c[:])


_CACHE = {}


def _build(T=T_FULL):
    if ("nc", T) in _CACHE:
        return _CACHE[("nc", T)]
    nc = bacc.Bacc("TRN2", target_bir_lowering=False, debug=False)
    zt = nc.dram_tensor("zt", [T, LATENT, B_CORE], F32, kind="ExternalInput").ap()
    w_z = nc.dram_tensor("w_z", [LATENT, 128], F32, kind="ExternalInput").ap()
    w_hz = nc.dram_tensor("w_hz", [HID + 1, 128], F32, kind="ExternalInput").ap()
    w_zp = nc.dram_tensor("w_zp", [HID + 1, 32], F32, kind="ExternalInput").ap()
    zp = nc.dram_tensor("zp", [T, LATENT, B_CORE], F32, kind="ExternalOutput").ap()
    hc = nc.dram_tensor("hc", [2, HID, B_CORE], F32, kind="ExternalOutput").ap()
    with tile.TileContext(nc) as tc:
        lstm_tile_kernel(tc, [zp, hc], [zt, w_z, w_hz, w_zp], T=T)
    nc.compile()
    _CACHE[("nc", T)] = nc
    return nc


def kernel(z_seq, W_ih, W_hh, b_ih, b_hh, W_out, b_out):
    z_seq = np.asarray(z_seq, dtype=np.float32)
    w_z, w_hz, w_zp = prep_weights(W_ih, W_hh, b_ih, b_hh, W_out, b_out)

    nc = _build()
    in_maps = []
    for c in range(NCORES):
        shard = z_seq[c * B_CORE : (c + 1) * B_CORE]  # [256, 512, 18]
        zt = np.ascontiguousarray(shard.transpose(1, 2, 0))  # [512, 18, 256]
        in_maps.append({"zt": zt, "w_z": w_z, "w_hz": w_hz, "w_zp": w_zp})

    trace = os.environ.get("KERNEL_TRACE", "0") == "1"
    res = run_bass_kernel_spmd(nc, in_maps, list(range(NCORES)), trace=trace)
    if trace and res.exec_time_ns is not None:
        print(f"HW exec time: {res.exec_time_ns} ns")

    zp_full = np.empty((NB, T_FULL, LATENT), np.float32)
    h_n = np.empty((NB, HID), np.float32)
    c_n = np.empty((NB, HID), np.float32)
    for c in range(NCORES):
        r = res.results[c]
        zp_full[c * B_CORE : (c + 1) * B_CORE] = r["zp"].transpose(2, 0, 1)
        h_n[c * B_CORE : (c + 1) * B_CORE] = r["hc"][0].T
        c_n[c * B_CORE : (c + 1) * B_CORE] = r["hc"][1].T
    return zp_full, (h_n[None], c_n[None])


# revision 8
# speedup vs baseline: 50.4510x; 50.4510x over previous
"""Self-contained Trainium2 kernel for nn_BaselineDynamics (LSTM + linear head).

kernel(**inputs) takes the FULL inputs (z_seq [2048,512,18], LSTM/linear
weights), shards the batch across 8 NeuronCores (pure data parallelism,
replicated weights), runs a Bass/Tile kernel per core, and returns the full
(z_pred, (h_n, c_n)) matching reference().

See lstm_tile_kernel docstring below for the device-side algorithm.
"""

import os
from contextlib import ExitStack

import numpy as np

import concourse.bass as bass
import concourse.bacc as bacc
import concourse.tile as tile
from concourse import mybir
from concourse._compat import with_exitstack
from concourse.bass_utils import run_bass_kernel_spmd

F32 = mybir.dt.float32

LATENT = 18
HID = 32
NB = 2048
T_FULL = 512
NCORES = 8
B_CORE = NB // NCORES  # 256

# torch gate order [i, f, g, o] -> device order [f, i, o, g]
# (f/o at slot base 0, i/g at slot base 32 so every elementwise op's two
# inputs share a partition base -- walrus requires equal bases for SB+SB ops)
PERM = np.concatenate(
    [np.arange(32, 64), np.arange(0, 32), np.arange(96, 128), np.arange(64, 96)]
)


def prep_weights(W_ih, W_hh, b_ih, b_hh, W_out, b_out):
    """Host-side weight folding.

    Returns lhsT matrices:
      w_z  [18, 128] : W_ih (permuted, g-rows doubled)
      w_hz [33, 128] : [2*W_hh (permuted, g-rows doubled); b_ih+b_hh (permuted, g-rows doubled)]
      w_zp [33, 32]  : [2*W_out; b_out] zero-padded to 32 columns
    """
    Wih_p = np.array(W_ih, dtype=np.float32)[PERM].copy()
    Whh_p = np.array(W_hh, dtype=np.float32)[PERM].copy()
    b_pv = (np.array(b_ih, dtype=np.float32) + np.array(b_hh, dtype=np.float32))[
        PERM
    ].copy()
    gsl = slice(96, 128)
    Wih_p[gsl] *= 2.0
    Whh_p[gsl] *= 2.0
    b_pv[gsl] *= 2.0
    Whh2 = 2.0 * Whh_p  # h = 2*h'
    w_z = np.ascontiguousarray(Wih_p.T)  # [18, 128]
    w_hz = np.ascontiguousarray(
        np.concatenate([Whh2.T, b_pv[None, :]], axis=0)
    )  # [33, 128]
    w_zp = np.concatenate(
        [
            (2.0 * np.array(W_out, dtype=np.float32)).T,
            np.array(b_out, dtype=np.float32)[None, :],
        ],
        axis=0,
    )  # [33, 18]
    w_zp = np.ascontiguousarray(
        np.concatenate([w_zp, np.zeros((HID + 1, 32 - LATENT), np.float32)], axis=1)
    )  # [33, 32] zero-padded so each PSUM slot is fully written
    return w_z, w_hz, w_zp


@with_exitstack
def lstm_tile_kernel(
    ctx: ExitStack, tc: tile.TileContext, outs, ins, T=T_FULL, B=B_CORE, M=1, reps=1
):
    """Gate-major LSTM.  Per step t:
      psum_g = W_z.T @ z_t + W_hz.T @ [h'; 1]      (PE)
      S      = sigmoid(psum_g)                      (ACT; tanh folded via 2sig(2x)-1)
      u      = (S_g - 0.5) * S_i                    (GPSIMD)  = i*g/2
      tmp    = S_f * c'; c' = tmp + u               (DVE)
      Tc     = sigmoid(4*c')                        (ACT)     = (tanh(c)+1)/2
      h'     = (Tc - 0.5) * S_o  -> stack[0:32]     (DVE)     = h/2
      zp     = W_zp.T @ [h'; 1]                     (PE, 4 steps per PSUM tile)
    """
    nc = tc.nc
    zp_out, hc_out = outs
    zt_in, w_z_in, w_hz_in, w_zp_in = ins
    BC = B // M

    const_pool = ctx.enter_context(tc.tile_pool(name="const", bufs=1))
    state_pool = ctx.enter_context(tc.tile_pool(name="state", bufs=1))
    zpool = ctx.enter_context(tc.tile_pool(name="zin", bufs=8))
    spool = ctx.enter_context(tc.tile_pool(name="sig", bufs=3))
    vpool = ctx.enter_context(tc.tile_pool(name="vec", bufs=3))
    gpsum = ctx.enter_context(tc.tile_pool(name="gps", bufs=3, space="PSUM"))
    zpsum = ctx.enter_context(tc.tile_pool(name="zps", bufs=3, space="PSUM"))

    wz_sb = const_pool.tile([LATENT, 128], F32, tag="wz")
    nc.sync.dma_start(wz_sb[:], w_z_in[:])
    whz_sb = const_pool.tile([HID + 1, 128], F32, tag="whz")
    nc.sync.dma_start(whz_sb[:], w_hz_in[:])
    wzp_sb = const_pool.tile([HID + 1, 32], F32, tag="wzp")
    nc.sync.dma_start(wzp_sb[:], w_zp_in[:])

    stack = state_pool.tile([HID + 1, B], F32, tag="stack")  # 0:32 h', row 32 ones
    cp = state_pool.tile([HID, B], F32, tag="cp")
    nc.vector.memset(stack[0:HID, :], 0.0)
    nc.vector.memset(stack[HID : HID + 1, :], 1.0)
    nc.vector.memset(cp[:], 0.0)

    SIG = mybir.ActivationFunctionType.Sigmoid

    rep_cm = tc.For_i(0, reps, 1) if reps > 1 else None
    if rep_cm is not None:
        ctx.enter_context(rep_cm)

    zps = None

    def emit_zp(t):
        # zp matmul for step t (stack still holds h'(t)); 4 steps share one
        # PSUM tile at partition slots 32*j, one copy+DMA per 4 steps.
        # Emitted one iteration late so it never blocks the next MM-h.
        nonlocal zps
        j = t % 4
        if j == 0:
            zps = zpsum.tile([128, B], F32, tag="zp")
        for m in range(M):
            cs = bass.ts(m, BC)
            nc.tensor.matmul(
                zps[32 * j : 32 * j + 32, cs], wzp_sb[:], stack[:, cs],
                start=True, stop=True, tile_position=(0, 32 * j),
            )
        if j == 3 or t == T - 1:
            zsb = spool.tile([128, B], F32, tag="zsb")
            nc.vector.tensor_copy(zsb[: 32 * (j + 1), :], zps[: 32 * (j + 1), :])
            for jj in range(j + 1):
                nc.sync.dma_start(
                    zp_out[t - j + jj], zsb[32 * jj : 32 * jj + LATENT, :]
                )

    for t in range(T):
        z_t = zpool.tile([LATENT, B], F32, tag="z")
        nc.sync.dma_start(z_t[:], zt_in[t])

        ps = gpsum.tile([128, B], F32, tag="g")
        for m in range(M):
            cs = bass.ts(m, BC)
            nc.tensor.matmul(ps[:, cs], wz_sb[:], z_t[:, cs], start=True, stop=False)
            nc.tensor.matmul(ps[:, cs], whz_sb[:], stack[:, cs], start=False, stop=True)

        if t > 0:
            emit_zp(t - 1)

        # S split into two base-0 tiles: s_lo rows {f, i}, s_hi rows {o, g}
        s_lo = spool.tile([64, B], F32, tag="Slo")
        s_hi = spool.tile([64, B], F32, tag="Shi")
        nc.scalar.activation(s_lo[:], ps[0:64, :], SIG)
        nc.scalar.activation(s_hi[:], ps[64:128, :], SIG)

        u = vpool.tile([HID, B], F32, tag="u")
        nc.vector.scalar_tensor_tensor(
            u[:], s_hi[32:64, :], 0.5, s_lo[32:64, :],
            op0=mybir.AluOpType.subtract, op1=mybir.AluOpType.mult,
        )
        tmp = vpool.tile([HID, B], F32, tag="tmp")
        nc.vector.tensor_mul(tmp[:], s_lo[0:32, :], cp[:])
        nc.vector.tensor_add(cp[:], tmp[:], u[:])

        Tc = vpool.tile([HID, B], F32, tag="Tc")
        nc.scalar.activation(Tc[:], cp[:], SIG, scale=4.0)
        nc.vector.scalar_tensor_tensor(
            stack[0:HID, :], Tc[:], 0.5, s_hi[0:32, :],
            op0=mybir.AluOpType.subtract, op1=mybir.AluOpType.mult,
        )

    emit_zp(T - 1)

    fin_h = vpool.tile([HID, B], F32, tag="fin")
    nc.scalar.mul(fin_h[:], stack[0:HID, :], 2.0)
    nc.sync.dma_start(hc_out[0], fin_h[:])
    fin_c = vpool.tile([HID, B], F32, tag="fin")
    nc.scalar.mul(fin_c[:], cp[:], 2.0)
    nc.sync.dma_start(hc_out[1], fin_c[:])


_CACHE = {}


def _build(T=T_FULL, reps=1):
    if ("nc", T, reps) in _CACHE:
        return _CACHE[("nc", T, reps)]
    nc = bacc.Bacc("TRN2", target_bir_lowering=False, debug=False)
    zt = nc.dram_tensor("zt", [T, LATENT, B_CORE], F32, kind="ExternalInput").ap()
    w_z = nc.dram_tensor("w_z", [LATENT, 128], F32, kind="ExternalInput").ap()
    w_hz = nc.dram_tensor("w_hz", [HID + 1, 128], F32, kind="ExternalInput").ap()
    w_zp = nc.dram_tensor("w_zp", [HID + 1, 32], F32, kind="ExternalInput").ap()
    zp = nc.dram_tensor("zp", [T, LATENT, B_CORE], F32, kind="ExternalOutput").ap()
    hc = nc.dram_tensor("hc", [2, HID, B_CORE], F32, kind="ExternalOutput").ap()
    with tile.TileContext(nc) as tc:
        lstm_tile_kernel(tc, [zp, hc], [zt, w_z, w_hz, w_zp], T=T, reps=reps)
    nc.compile()
    _CACHE[("nc", T, reps)] = nc
    return nc


def kernel(z_seq, W_ih, W_hh, b_ih, b_hh, W_out, b_out):
    z_seq = np.asarray(z_seq, dtype=np.float32)
    w_z, w_hz, w_zp = prep_weights(W_ih, W_hh, b_ih, b_hh, W_out, b_out)

    nc = _build()
    in_maps = []
    for c in range(NCORES):
        shard = z_seq[c * B_CORE : (c + 1) * B_CORE]  # [256, 512, 18]
        zt = np.ascontiguousarray(shard.transpose(1, 2, 0))  # [512, 18, 256]
        in_maps.append({"zt": zt, "w_z": w_z, "w_hz": w_hz, "w_zp": w_zp})

    trace = os.environ.get("KERNEL_TRACE", "0") == "1"
    res = run_bass_kernel_spmd(nc, in_maps, list(range(NCORES)), trace=trace)
    if trace and res.exec_time_ns is not None:
        print(f"HW exec time: {res.exec_time_ns} ns")

    zp_full = np.empty((NB, T_FULL, LATENT), np.float32)
    h_n = np.empty((NB, HID), np.float32)
    c_n = np.empty((NB, HID), np.float32)
    for c in range(NCORES):
        r = res.results[c]
        zp_full[c * B_CORE : (c + 1) * B_CORE] = r["zp"].transpose(2, 0, 1)
        h_n[c * B_CORE : (c + 1) * B_CORE] = r["hc"][0].T
        c_n[c * B_CORE : (c + 1) * B_CORE] = r["hc"][1].T
    return zp_full, (h_n[None], c_n[None])
